# revision 6
# baseline (speedup 1.0000x reference)
"""Dilated-attention Trainium2 kernel (8 NeuronCores, SPMD), bf16 edition.

Problem: x [4, 16384, 768] f32. Per 512-token segment, take every 2nd
position (dilation 2) -> 128 independent segments of [256, 768]; per-segment
self-attention out = softmax(xs @ xs.T / sqrt(768)) @ xs; output [4, 8192, 768].

Sharding: 128 (batch x segment) attention problems are fully independent ->
16 segments per core, no cross-core communication. Dilation gather, bf16
cast and the position-major -> partition-major permutation are done host-side
while building each core's input slice (pure data movement; bf16 keeps the
overall relative error ~1.7e-3, well under the 2e-2 gate).

Device layout: DRAM x is staged as [128 p, 16 s, 2 t, 772] bf16 where
position = t*128 + p and columns 768:772 hold literal 1.0 (fused softmax
denominator); y is [128 p, 16 s, 2 t, 768] bf16. All DMAs are therefore
per-partition contiguous (12KB+ lines per 4-segment batch).

Per segment (L=256 positions, D=768 features):
  1. input DMA (sync/HWDGE ring), batch of 4 segments
  2. feature-major XT via hardware xbar DMA-transpose (scalar/HWDGE ring),
     one [128,768]->[128,6,128] transpose per position tile (VARIANT=xbar),
     or PE transposes + DVE/ACT evicts (VARIANT=pe)
  3. S^T tiles in PSUM f32: kt0 full [128,256] (6 bf16 matmuls over d),
     kt1 only q 128:256 [128,128] -- S is symmetric, the missing corner
     E_B[:, 0:128] is the PE-transpose of E_A[:, 128:256] after exp
  4. exp on ScalarE with scale 1/sqrt(768) -> E bf16
  5. out tiles [128, 384|388] f32 = E[kt][:, qblk].T @ [X[kt] | ones]
     accumulated over kt; ones columns make op1[:, 384] the denominator
  6. recip (DVE) + normalize-evict PSUM->SBUF bf16 (split ScalarE/DVE)
  7. output DMA (gpsimd/SWDGE), batch of 4 segments
"""

import numpy as np
import ml_dtypes

import concourse.bass as bass
import concourse.mybir as mybir
import concourse.tile as tile
from concourse.bass_utils import run_bass_kernel_spmd
from concourse.masks import make_identity

F32 = mybir.dt.float32
BF16 = mybir.dt.bfloat16

B, S_FULL, D = 4, 16384, 768
SEG, DIL = 512, 2
L = SEG // DIL                      # 256 positions per dilated segment
NSEG = B * (S_FULL // SEG)          # 128 segments total
NCORE = 8
SEG_PER_CORE = NSEG // NCORE        # 16
KT = L // 128                       # 2 position tiles per segment
DT = D // 128                       # 6 feature tiles
DW = D + 4                          # input free pitch (cols 768:772 = 1.0)
SCALE = 1.0 / float(np.sqrt(D))
MAXB = 4                            # segments per DMA batch
TT = MAXB * KT                      # position tiles per batch buffer

VARIANT = "xbar"                    # "xbar" | "pe"


def build_nc(variant=VARIANT):
    nc = bass.Bass()
    x = nc.dram_tensor("x", [128, SEG_PER_CORE, KT, DW], BF16, kind="ExternalInput")
    y = nc.dram_tensor("y", [128, SEG_PER_CORE, KT, D], BF16, kind="ExternalOutput")
    Exp = mybir.ActivationFunctionType.Exp
    Copy = mybir.ActivationFunctionType.Copy

    with tile.TileContext(nc) as tc:
        with (
            tc.tile_pool(name="const", bufs=1) as const_pool,
            tc.tile_pool(name="xn", bufs=2) as xn_pool,
            tc.tile_pool(name="xt", bufs=3) as xt_pool,
            tc.tile_pool(name="e", bufs=16) as e_pool,
            tc.tile_pool(name="recip", bufs=8) as recip_pool,
            tc.tile_pool(name="osb", bufs=2) as osb_pool,
            tc.tile_pool(name="ps", bufs=2, space="PSUM") as ps_pool,
        ):
            identity_f = const_pool.tile([128, 128], F32)
            make_identity(nc, identity_f[:])
            identity = const_pool.tile([128, 128], BF16)
            nc.vector.tensor_copy(identity[:], identity_f[:])

            batches = [(0, 1), (1, 1), (2, 2), (4, 4), (8, 4), (12, 4)]
            for s0, bn in batches:
                TB = bn * KT
                xn = xn_pool.tile([128, TT, DW], BF16, tag="xn")
                nc.sync.dma_start(
                    out=xn[:, 0:TB, :],
                    in_=x[:, s0 : s0 + bn].rearrange("p s t d -> p (s t) d"),
                )

                # ---- Q/K phase for the whole batch
                es_all = []
                for sl in range(bn):
                    xt = xt_pool.tile([128, DT, KT, 128], BF16)
                    if variant == "xbar":
                        for t in range(KT):
                            nc.scalar.dma_start_transpose(
                                out=xt[:, :, t, :],
                                in_=xn[:, sl * KT + t, 0:D],
                            )
                    else:
                        for d in range(DT):
                            tp = ps_pool.tile([128, 256], BF16, tag="tp")
                            for t in range(KT):
                                nc.tensor.transpose(
                                    tp[:, t * 128 : (t + 1) * 128],
                                    xn[:, sl * KT + t, d * 128 : (d + 1) * 128],
                                    identity[:],
                                )
                            if d % 3 == 2:
                                nc.scalar.copy(xt[:, d], tp[:])
                            else:
                                nc.vector.tensor_copy(xt[:, d], tp[:])

                    sp = ps_pool.tile([128, 384], F32, tag="sp")
                    for d in range(DT):
                        nc.tensor.matmul(
                            sp[:, 0:256],
                            xt[:, d, 0, :],
                            xt[:, d],
                            start=(d == 0),
                            stop=(d == DT - 1),
                        )
                    for d in range(DT):
                        nc.tensor.matmul(
                            sp[:, 256:384],
                            xt[:, d, 1, :],
                            xt[:, d, 1, :],
                            start=(d == 0),
                            stop=(d == DT - 1),
                            skip_group_check=True,
                        )
                    ea = e_pool.tile([128, 256], BF16)
                    eb = e_pool.tile([128, 256], BF16)
                    nc.scalar.activation(ea[:], sp[:, 0:256], Exp, scale=SCALE)
                    nc.scalar.activation(
                        eb[:, 128:256], sp[:, 256:384], Exp, scale=SCALE
                    )
                    es_all.append((ea, eb))

                # ---- V phase: out matmuls + normalize + store per segment
                osb = osb_pool.tile([128, TT, D], BF16, tag="osb")
                for sl in range(bn):
                    ea, eb = es_all[sl]
                    # S symmetry: E_B[:, 0:128] = (E_A[:, 128:256]).T
                    tpe = ps_pool.tile([128, 128], BF16, tag="tp")
                    nc.tensor.transpose(tpe[:], ea[:, 128:256], identity[:])
                    nc.vector.tensor_copy(eb[:, 0:128], tpe[:])

                    for qt in (1, 0):  # qt=0 last: it needs the eb evict above
                        op0 = ps_pool.tile([128, 388], F32, tag="op0")
                        op1 = ps_pool.tile([128, 388], F32, tag="op1")
                        for kt in range(KT):
                            e = (ea, eb)[kt]
                            lhsT = e[:, qt * 128 : (qt + 1) * 128]
                            nc.tensor.matmul(
                                op0[:, 0:384],
                                lhsT,
                                xn[:, sl * KT + kt, 0:384],
                                start=(kt == 0),
                                stop=(kt == KT - 1),
                            )
                            nc.tensor.matmul(
                                op1[:, 0:388],
                                lhsT,
                                xn[:, sl * KT + kt, 384:772],
                                start=(kt == 0),
                                stop=(kt == KT - 1),
                            )
                        recip = recip_pool.tile([128, 1], F32)
                        nc.vector.reciprocal(recip[:], op1[:, 384:385])
                        dst = osb[:, sl * KT + qt]
                        nc.scalar.activation(
                            dst[:, 0:384], op0[:, 0:384], Copy, scale=recip[:]
                        )
                        nc.vector.tensor_scalar_mul(
                            dst[:, 384:768], op1[:, 0:384], recip[:]
                        )

                nc.gpsimd.dma_start(
                    out=y[:, s0 : s0 + bn].rearrange("p s t d -> p (s t) d"),
                    in_=osb[:, 0:TB],
                )
    return nc


def split_excess_waits(nc, max_waits=1):
    """This walrus build only encodes one sync wait per instruction; move
    excess waits onto preceding same-engine NOPs."""
    n_split = 0
    for fn in nc.m.functions:
        for blk in fn.blocks:
            insts = blk.instructions
            i = 0
            while i < len(insts):
                inst = insts[i]
                si = getattr(inst, "sync_info", None)
                waits = list(si.on_wait) if si and si.on_wait else []
                if len(waits) > max_waits:
                    nop = mybir.InstNoOp(name=f"I-waitsplit-{n_split}", ins=[], outs=[])
                    nop.engine = inst.engine
                    nop.sync_info = mybir.SyncInfo(
                        on_wait=waits[:max_waits], on_update=[]
                    )
                    inst.sync_info = mybir.SyncInfo(
                        on_wait=waits[max_waits:], on_update=list(si.on_update)
                    )
                    insts.insert(i, nop)
                    n_split += 1
                else:
                    i += 1
    return n_split


_NC = None


def _get_nc():
    global _NC
    if _NC is None:
        _NC = build_nc()
        split_excess_waits(_NC)
    return _NC


def shard_inputs(x):
    """Full x [4, 16384, 768] f32 -> 8 per-core dicts of [128, 16, 2, 772] bf16.

    Per-core layout: [p, s_local, t, d] with position = t*128 + p within the
    dilated segment, cols 768:772 = 1.0 (softmax denominator trick).
    """
    xd = np.asarray(x).reshape(B, S_FULL // SEG, SEG, D)[:, :, ::DIL, :]
    xd = xd.reshape(NSEG, KT, 128, D)           # [seg, t, p, d]
    xp = xd.transpose(2, 0, 1, 3)               # [p, seg, t, d]
    xb = np.empty((128, NSEG, KT, DW), dtype=ml_dtypes.bfloat16)
    xb[..., 0:D] = xp.astype(ml_dtypes.bfloat16)
    xb[..., D:DW] = np.asarray(1.0, dtype=ml_dtypes.bfloat16)
    return [
        {"x": np.ascontiguousarray(xb[:, SEG_PER_CORE * c : SEG_PER_CORE * (c + 1)])}
        for c in range(NCORE)
    ]


def assemble_output(results):
    ys = np.concatenate(
        [results[c]["y"] for c in range(NCORE)], axis=1
    )                                            # [p, seg, t, d] bf16
    out = ys.astype(np.float32).transpose(1, 2, 0, 3)  # [seg, t, p, d]
    return np.ascontiguousarray(out.reshape(B, (S_FULL // SEG) * L, D))


def kernel(x):
    nc = _get_nc()
    in_maps = shard_inputs(x)
    core_ids = list(range(NCORE))
    # run twice: the first execution after a fresh NEFF load has been seen
    # returning unwritten output buffers; the repeat is cheap and reliable.
    run_bass_kernel_spmd(nc, in_maps, core_ids)
    res = run_bass_kernel_spmd(nc, in_maps, core_ids)
    return assemble_output(res.results)


# revision 7
# speedup vs baseline: 1.4256x; 1.4256x over previous
"""Dilated-attention Trainium2 kernel (8 NeuronCores, SPMD), bf16 edition.

Problem: x [4, 16384, 768] f32. Per 512-token segment, take every 2nd
position (dilation 2) -> 128 independent segments of [256, 768]; per-segment
self-attention out = softmax(xs @ xs.T / sqrt(768)) @ xs; output [4, 8192, 768].

Sharding: 128 (batch x segment) attention problems are fully independent ->
16 segments per core, no cross-core communication. Dilation gather, bf16
cast and the position-major -> partition-major permutation are done host-side
while building each core's input slice (pure data movement; bf16 keeps the
overall relative error ~1.7e-3, well under the 2e-2 gate).

Device layout: DRAM x is staged as [128 p, 16 s, 2 t, 772] bf16 where
position = t*128 + p and columns 768:772 hold literal 1.0 (fused softmax
denominator); y is [128 p, 16 s, 2 t, 768] bf16. All DMAs are therefore
per-partition contiguous (12KB+ lines per 4-segment batch).

Per segment (L=256 positions, D=768 features):
  1. input DMA (sync/HWDGE ring), batch of 4 segments
  2. feature-major XT via hardware xbar DMA-transpose (scalar/HWDGE ring),
     one [128,768]->[128,6,128] transpose per position tile (VARIANT=xbar),
     or PE transposes + DVE/ACT evicts (VARIANT=pe)
  3. S^T tiles in PSUM f32: kt0 full [128,256] (6 bf16 matmuls over d),
     kt1 only q 128:256 [128,128] -- S is symmetric, the missing corner
     E_B[:, 0:128] is the PE-transpose of E_A[:, 128:256] after exp
  4. exp on ScalarE with scale 1/sqrt(768) -> E bf16
  5. out tiles [128, 384|388] f32 = E[kt][:, qblk].T @ [X[kt] | ones]
     accumulated over kt; ones columns make op1[:, 384] the denominator
  6. recip (DVE) + normalize-evict PSUM->SBUF bf16 (split ScalarE/DVE)
  7. output DMA (gpsimd/SWDGE), batch of 4 segments
"""

import numpy as np
import ml_dtypes

import concourse.bass as bass
import concourse.mybir as mybir
import concourse.tile as tile
from concourse.bass_utils import run_bass_kernel_spmd
from concourse.masks import make_identity

F32 = mybir.dt.float32
BF16 = mybir.dt.bfloat16

B, S_FULL, D = 4, 16384, 768
SEG, DIL = 512, 2
L = SEG // DIL                      # 256 positions per dilated segment
NSEG = B * (S_FULL // SEG)          # 128 segments total
NCORE = 8
SEG_PER_CORE = NSEG // NCORE        # 16
KT = L // 128                       # 2 position tiles per segment
DT = D // 128                       # 6 feature tiles
DW = D + 4                          # input free pitch (cols 768:772 = 1.0)
SCALE = 1.0 / float(np.sqrt(D))
MAXB = 4                            # segments per DMA batch
TT = MAXB * KT                      # position tiles per batch buffer

VARIANT = "pe"                      # "xbar" | "pe"


def build_nc(variant=VARIANT):
    nc = bass.Bass()
    x = nc.dram_tensor("x", [128, SEG_PER_CORE, KT, DW], BF16, kind="ExternalInput")
    y = nc.dram_tensor("y", [128, SEG_PER_CORE, KT, D], BF16, kind="ExternalOutput")
    Exp = mybir.ActivationFunctionType.Exp
    Copy = mybir.ActivationFunctionType.Copy

    with tile.TileContext(nc) as tc:
        with (
            tc.tile_pool(name="const", bufs=1) as const_pool,
            tc.tile_pool(name="xn", bufs=2) as xn_pool,
            tc.tile_pool(name="xt", bufs=3) as xt_pool,
            tc.tile_pool(name="e", bufs=16) as e_pool,
            tc.tile_pool(name="recip", bufs=8) as recip_pool,
            tc.tile_pool(name="osb", bufs=2) as osb_pool,
            tc.tile_pool(name="ps", bufs=2, space="PSUM") as ps_pool,
        ):
            identity_f = const_pool.tile([128, 128], F32)
            make_identity(nc, identity_f[:])
            identity = const_pool.tile([128, 128], BF16)
            nc.vector.tensor_copy(identity[:], identity_f[:])

            batches = [(0, 1), (1, 1), (2, 2), (4, 4), (8, 4), (12, 4)]
            for s0, bn in batches:
                TB = bn * KT
                xn = xn_pool.tile([128, TT, DW], BF16, tag="xn")
                nc.sync.dma_start(
                    out=xn[:, 0:TB, :],
                    in_=x[:, s0 : s0 + bn].rearrange("p s t d -> p (s t) d"),
                )

                # ---- Q/K phase for the whole batch
                es_all = []
                for sl in range(bn):
                    xt = xt_pool.tile([128, DT, KT, 128], BF16)
                    if variant == "xbar":
                        for t in range(KT):
                            nc.scalar.dma_start_transpose(
                                out=xt[:, :, t, :],
                                in_=xn[:, sl * KT + t, 0:D],
                            )
                    else:
                        for d in range(DT):
                            tp = ps_pool.tile([128, 256], BF16, tag="tp")
                            for t in range(KT):
                                nc.tensor.transpose(
                                    tp[:, t * 128 : (t + 1) * 128],
                                    xn[:, sl * KT + t, d * 128 : (d + 1) * 128],
                                    identity[:],
                                )
                            if d % 3 == 2:
                                nc.scalar.copy(xt[:, d], tp[:])
                            else:
                                nc.vector.tensor_copy(xt[:, d], tp[:])

                    sp = ps_pool.tile([128, 384], F32, tag="sp")
                    for d in range(DT):
                        nc.tensor.matmul(
                            sp[:, 0:256],
                            xt[:, d, 0, :],
                            xt[:, d],
                            start=(d == 0),
                            stop=(d == DT - 1),
                        )
                    for d in range(DT):
                        nc.tensor.matmul(
                            sp[:, 256:384],
                            xt[:, d, 1, :],
                            xt[:, d, 1, :],
                            start=(d == 0),
                            stop=(d == DT - 1),
                            skip_group_check=True,
                        )
                    ea = e_pool.tile([128, 256], BF16)
                    eb = e_pool.tile([128, 256], BF16)
                    nc.scalar.activation(ea[:], sp[:, 0:256], Exp, scale=SCALE)
                    nc.scalar.activation(
                        eb[:, 128:256], sp[:, 256:384], Exp, scale=SCALE
                    )
                    es_all.append((ea, eb))

                # ---- V phase: out matmuls + normalize + store per segment
                osb = osb_pool.tile([128, TT, D], BF16, tag="osb")
                for sl in range(bn):
                    ea, eb = es_all[sl]
                    # S symmetry: E_B[:, 0:128] = (E_A[:, 128:256]).T
                    tpe = ps_pool.tile([128, 128], BF16, tag="tp")
                    nc.tensor.transpose(tpe[:], ea[:, 128:256], identity[:])
                    nc.vector.tensor_copy(eb[:, 0:128], tpe[:])

                    for qt in (1, 0):  # qt=0 last: it needs the eb evict above
                        op0 = ps_pool.tile([128, 388], F32, tag="op0")
                        op1 = ps_pool.tile([128, 388], F32, tag="op1")
                        for kt in range(KT):
                            e = (ea, eb)[kt]
                            lhsT = e[:, qt * 128 : (qt + 1) * 128]
                            nc.tensor.matmul(
                                op0[:, 0:384],
                                lhsT,
                                xn[:, sl * KT + kt, 0:384],
                                start=(kt == 0),
                                stop=(kt == KT - 1),
                            )
                            nc.tensor.matmul(
                                op1[:, 0:388],
                                lhsT,
                                xn[:, sl * KT + kt, 384:772],
                                start=(kt == 0),
                                stop=(kt == KT - 1),
                            )
                        recip = recip_pool.tile([128, 1], F32)
                        nc.vector.reciprocal(recip[:], op1[:, 384:385])
                        dst = osb[:, sl * KT + qt]
                        nc.scalar.activation(
                            dst[:, 0:384], op0[:, 0:384], Copy, scale=recip[:]
                        )
                        nc.vector.tensor_scalar_mul(
                            dst[:, 384:768], op1[:, 0:384], recip[:]
                        )

                nc.gpsimd.dma_start(
                    out=y[:, s0 : s0 + bn].rearrange("p s t d -> p (s t) d"),
                    in_=osb[:, 0:TB],
                )
    return nc


def split_excess_waits(nc, max_waits=1):
    """This walrus build only encodes one sync wait per instruction; move
    excess waits onto preceding same-engine NOPs."""
    n_split = 0
    for fn in nc.m.functions:
        for blk in fn.blocks:
            insts = blk.instructions
            i = 0
            while i < len(insts):
                inst = insts[i]
                si = getattr(inst, "sync_info", None)
                waits = list(si.on_wait) if si and si.on_wait else []
                if len(waits) > max_waits:
                    nop = mybir.InstNoOp(name=f"I-waitsplit-{n_split}", ins=[], outs=[])
                    nop.engine = inst.engine
                    nop.sync_info = mybir.SyncInfo(
                        on_wait=waits[:max_waits], on_update=[]
                    )
                    inst.sync_info = mybir.SyncInfo(
                        on_wait=waits[max_waits:], on_update=list(si.on_update)
                    )
                    insts.insert(i, nop)
                    n_split += 1
                else:
                    i += 1
    return n_split


_NC = None


def _get_nc():
    global _NC
    if _NC is None:
        _NC = build_nc()
        split_excess_waits(_NC)
    return _NC


def shard_inputs(x):
    """Full x [4, 16384, 768] f32 -> 8 per-core dicts of [128, 16, 2, 772] bf16.

    Per-core layout: [p, s_local, t, d] with position = t*128 + p within the
    dilated segment, cols 768:772 = 1.0 (softmax denominator trick).
    """
    xd = np.asarray(x).reshape(B, S_FULL // SEG, SEG, D)[:, :, ::DIL, :]
    xd = xd.reshape(NSEG, KT, 128, D)           # [seg, t, p, d]
    xp = xd.transpose(2, 0, 1, 3)               # [p, seg, t, d]
    xb = np.empty((128, NSEG, KT, DW), dtype=ml_dtypes.bfloat16)
    xb[..., 0:D] = xp.astype(ml_dtypes.bfloat16)
    xb[..., D:DW] = np.asarray(1.0, dtype=ml_dtypes.bfloat16)
    return [
        {"x": np.ascontiguousarray(xb[:, SEG_PER_CORE * c : SEG_PER_CORE * (c + 1)])}
        for c in range(NCORE)
    ]


def assemble_output(results):
    ys = np.concatenate(
        [results[c]["y"] for c in range(NCORE)], axis=1
    )                                            # [p, seg, t, d] bf16
    out = ys.astype(np.float32).transpose(1, 2, 0, 3)  # [seg, t, p, d]
    return np.ascontiguousarray(out.reshape(B, (S_FULL // SEG) * L, D))


def kernel(x):
    nc = _get_nc()
    in_maps = shard_inputs(x)
    core_ids = list(range(NCORE))
    # run twice: the first execution after a fresh NEFF load has been seen
    # returning unwritten output buffers; the repeat is cheap and reliable.
    run_bass_kernel_spmd(nc, in_maps, core_ids)
    res = run_bass_kernel_spmd(nc, in_maps, core_ids)
    return assemble_output(res.results)


# revision 8
# speedup vs baseline: 1.7392x; 1.2200x over previous
"""Dilated-attention Trainium2 kernel (8 NeuronCores, SPMD), bf16/fp8 edition.

Problem: x [4, 16384, 768] f32. Per 512-token segment, take every 2nd
position (dilation 2) -> 128 independent segments of [256, 768]; per-segment
self-attention out = softmax(xs @ xs.T / sqrt(768)) @ xs; output [4, 8192, 768].

Sharding: 128 (batch x segment) attention problems are fully independent ->
16 segments per core, no cross-core communication. The dilation gather, the
position-major -> partition-major permutation, the bf16/fp8 casts and the
final numerator/denominator divide are host-side (pure data movement /
elementwise; overall relative error ~2.3e-3, well under the 2e-2 gate).

Device inputs per core (all per-partition contiguous in DRAM):
  x   [128 p, 16 s, 2 t, 772] bf16 -- position-major, position = t*128+p,
      cols 768:772 hold literal 1.0 (fused softmax denominator)
  xt  [128 dp, 16 s, 6 dc, 2 t, 128 pc] fp8e4m3 -- feature-major transposed
      copy (feature = dc*128+dp), Q/K side only; fp8 only perturbs logits
Output y [128 p, 16 s, 2 t, 772] bf16: cols 0:768 = un-normalized E @ [X|1]
numerator, col 768 = softmax denominator; host divides.

Per segment (L=256, D=768):
  1. batch input DMAs (x on sync HWDGE ring, xt on scalar HWDGE ring)
  2. S^T tiles in PSUM f32 from fp8 matmuls: kt0 full [128,256], kt1 only
     q 128:256 -- S is symmetric, the missing corner E_B[:, 0:128] is the
     PE-transpose of E_A[:, 128:256] after exp
  3. one exp per segment on ScalarE (scale 1/sqrt(768)) -> E bf16 [128,384]
  4. out tiles [128, 384|388] f32 = E[kt][:, qblk].T @ [X[kt] | ones] bf16
  5. plain PSUM->SBUF bf16 evicts (split ScalarE/VectorE), no normalize
  6. per-segment output DMA (gpsimd SWDGE) -- keeps the pipeline tail short
"""

import numpy as np
import ml_dtypes

import concourse.bass as bass
import concourse.mybir as mybir
import concourse.tile as tile
from concourse.bass_utils import run_bass_kernel_spmd
from concourse.masks import make_identity

F32 = mybir.dt.float32
BF16 = mybir.dt.bfloat16
FP8 = mybir.dt.float8e4

B, S_FULL, D = 4, 16384, 768
SEG, DIL = 512, 2
L = SEG // DIL                      # 256 positions per dilated segment
NSEG = B * (S_FULL // SEG)          # 128 segments total
NCORE = 8
SEG_PER_CORE = NSEG // NCORE        # 16
KT = L // 128                       # 2 position tiles per segment
DT = D // 128                       # 6 feature tiles
DW = D + 4                          # free pitch (cols 768:772 = 1.0)
SCALE = 1.0 / float(np.sqrt(D))
MAXB = 4                            # segments per input-DMA batch
TT = MAXB * KT


def build_nc():
    nc = bass.Bass()
    x = nc.dram_tensor("x", [128, SEG_PER_CORE, KT, DW], BF16, kind="ExternalInput")
    xt = nc.dram_tensor(
        "xt", [128, SEG_PER_CORE, DT, KT, 128], FP8, kind="ExternalInput"
    )
    y = nc.dram_tensor("y", [128, SEG_PER_CORE, KT, DW], BF16, kind="ExternalOutput")
    Exp = mybir.ActivationFunctionType.Exp

    with tile.TileContext(nc) as tc:
        with (
            tc.tile_pool(name="const", bufs=1) as const_pool,
            tc.tile_pool(name="xn", bufs=2) as xn_pool,
            tc.tile_pool(name="xf", bufs=2) as xf_pool,
            tc.tile_pool(name="e", bufs=8) as e_pool,
            tc.tile_pool(name="osb", bufs=4) as osb_pool,
            tc.tile_pool(name="ps", bufs=2, space="PSUM") as ps_pool,
        ):
            identity_f = const_pool.tile([128, 128], F32)
            make_identity(nc, identity_f[:])
            identity = const_pool.tile([128, 128], BF16)
            nc.vector.tensor_copy(identity[:], identity_f[:])

            batches = [(0, 1), (1, 1), (2, 2), (4, 4), (8, 4), (12, 4)]
            for s0, bn in batches:
                TB = bn * KT
                xn = xn_pool.tile([128, TT, DW], BF16, tag="xn")
                xf = xf_pool.tile([128, MAXB, DT, KT, 128], FP8, tag="xf")
                nc.scalar.dma_start(
                    out=xf[:, 0:bn],
                    in_=xt[:, s0 : s0 + bn],
                )
                nc.sync.dma_start(
                    out=xn[:, 0:TB, :],
                    in_=x[:, s0 : s0 + bn].rearrange("p s t d -> p (s t) d"),
                )

                # ---- Q/K phase for the whole batch
                es_all = []
                for sl in range(bn):
                    sp = ps_pool.tile([128, 384], F32, tag="sp")
                    for d in range(DT):
                        nc.tensor.matmul(
                            sp[:, 0:256],
                            xf[:, sl, d, 0, :],
                            xf[:, sl, d],
                            start=(d == 0),
                            stop=(d == DT - 1),
                        )
                    for d in range(DT):
                        nc.tensor.matmul(
                            sp[:, 256:384],
                            xf[:, sl, d, 1, :],
                            xf[:, sl, d, 1, :],
                            start=(d == 0),
                            stop=(d == DT - 1),
                            skip_group_check=True,
                        )
                    # e[:, 0:256] = E_A; e[:, 256:384] = E_B[:, 128:256],
                    # e[:, 384:512] = E_B[:, 0:128] (transposed corner)
                    e = e_pool.tile([128, 512], BF16)
                    nc.scalar.activation(e[:, 0:384], sp[:], Exp, scale=SCALE)
                    es_all.append(e)

                # ---- V phase + store per segment
                for sl in range(bn):
                    e = es_all[sl]
                    # S symmetry: E_B[:, 0:128] = (E_A[:, 128:256]).T
                    tpe = ps_pool.tile([128, 128], BF16, tag="tp")
                    nc.tensor.transpose(tpe[:], e[:, 128:256], identity[:])
                    nc.vector.tensor_copy(e[:, 384:512], tpe[:])

                    osb = osb_pool.tile([128, KT, DW], BF16)
                    for qt in (1, 0):  # qt=0 last: it needs the corner evict
                        op0 = ps_pool.tile([128, 388], F32, tag="op0")
                        op1 = ps_pool.tile([128, 388], F32, tag="op1")
                        for kt in range(KT):
                            if kt == 0:
                                lhsT = e[:, qt * 128 : qt * 128 + 128]
                            elif qt == 1:
                                lhsT = e[:, 256:384]
                            else:
                                lhsT = e[:, 384:512]
                            nc.tensor.matmul(
                                op0[:, 0:384],
                                lhsT,
                                xn[:, sl * KT + kt, 0:384],
                                start=(kt == 0),
                                stop=(kt == KT - 1),
                            )
                            nc.tensor.matmul(
                                op1[:, 0:388],
                                lhsT,
                                xn[:, sl * KT + kt, 384:772],
                                start=(kt == 0),
                                stop=(kt == KT - 1),
                            )
                        dst = osb[:, qt]
                        if qt:
                            nc.scalar.copy(dst[:, 0:384], op0[:, 0:384])
                            nc.vector.tensor_copy(dst[:, 384:772], op1[:])
                        else:
                            nc.vector.tensor_copy(dst[:, 0:384], op0[:, 0:384])
                            nc.scalar.copy(dst[:, 384:772], op1[:])

                    nc.gpsimd.dma_start(
                        out=y[:, s0 + sl].rearrange("p t d -> p (t d)"),
                        in_=osb[:].rearrange("p t d -> p (t d)"),
                    )
    return nc


def split_excess_waits(nc, max_waits=1):
    """This walrus build only encodes one sync wait per instruction; move
    excess waits onto preceding same-engine NOPs."""
    n_split = 0
    for fn in nc.m.functions:
        for blk in fn.blocks:
            insts = blk.instructions
            i = 0
            while i < len(insts):
                inst = insts[i]
                si = getattr(inst, "sync_info", None)
                waits = list(si.on_wait) if si and si.on_wait else []
                if len(waits) > max_waits:
                    nop = mybir.InstNoOp(name=f"I-waitsplit-{n_split}", ins=[], outs=[])
                    nop.engine = inst.engine
                    nop.sync_info = mybir.SyncInfo(
                        on_wait=waits[:max_waits], on_update=[]
                    )
                    inst.sync_info = mybir.SyncInfo(
                        on_wait=waits[max_waits:], on_update=list(si.on_update)
                    )
                    insts.insert(i, nop)
                    n_split += 1
                else:
                    i += 1
    return n_split


_NC = None


def _get_nc():
    global _NC
    if _NC is None:
        _NC = build_nc()
        split_excess_waits(_NC)
    return _NC


def shard_inputs(x):
    """Full x [4, 16384, 768] f32 -> 8 per-core dicts:
    x  [128, 16, 2, 772] bf16 (position-major + ones cols)
    xt [128, 16, 6, 2, 128] fp8e4m3 (feature-major)
    """
    xd = np.asarray(x).reshape(B, S_FULL // SEG, SEG, D)[:, :, ::DIL, :]
    xd = xd.reshape(NSEG, KT, 128, D)                 # [seg, t, p, d]
    xp = xd.transpose(2, 0, 1, 3)                     # [p, seg, t, d]
    xb = np.empty((128, NSEG, KT, DW), dtype=ml_dtypes.bfloat16)
    xb[..., 0:D] = xp.astype(ml_dtypes.bfloat16)
    xb[..., D:DW] = np.asarray(1.0, dtype=ml_dtypes.bfloat16)
    xt = (
        xb[..., 0:D]
        .reshape(128, NSEG, KT, DT, 128)              # [p, seg, t, dc, dp]
        .transpose(4, 1, 3, 2, 0)                     # [dp, seg, dc, t, p]
        .astype(ml_dtypes.float8_e4m3)
    )
    out = []
    for c in range(NCORE):
        sl = slice(SEG_PER_CORE * c, SEG_PER_CORE * (c + 1))
        out.append(
            {
                "x": np.ascontiguousarray(xb[:, sl]),
                "xt": np.ascontiguousarray(xt[:, sl]),
            }
        )
    return out


def assemble_output(results):
    ys = np.concatenate([results[c]["y"] for c in range(NCORE)], axis=1)
    ys = ys.astype(np.float32)                        # [p, seg, t, 772]
    num = ys[..., 0:D].transpose(1, 2, 0, 3)          # [seg, t, p, d]
    den = ys[..., D].transpose(1, 2, 0)[..., None]    # [seg, t, p, 1]
    out = num / den
    return np.ascontiguousarray(out.reshape(B, (S_FULL // SEG) * L, D))


def kernel(x):
    nc = _get_nc()
    in_maps = shard_inputs(x)
    core_ids = list(range(NCORE))
    # run twice: the first execution after a fresh NEFF load has been seen
    # returning unwritten output buffers; the repeat is cheap and reliable.
    run_bass_kernel_spmd(nc, in_maps, core_ids)
    res = run_bass_kernel_spmd(nc, in_maps, core_ids)
    return assemble_output(res.results)


# revision 14
# speedup vs baseline: 1.8663x; 1.0731x over previous
"""Dilated-attention Trainium2 kernel (8 NeuronCores, SPMD), bf16/fp8 edition.

Problem: x [4, 16384, 768] f32. Per 512-token segment, take every 2nd
position (dilation 2) -> 128 independent segments of [256, 768]; per-segment
self-attention out = softmax(xs @ xs.T / sqrt(768)) @ xs; output [4, 8192, 768].

Sharding: 128 (batch x segment) attention problems are fully independent ->
16 segments per core, no cross-core communication. The dilation gather, the
position-major -> partition-major permutation, the bf16/fp8 casts and the
final numerator/denominator divide are host-side (pure data movement /
elementwise; overall relative error ~2.3e-3, well under the 2e-2 gate).

Device inputs per core (all per-partition contiguous in DRAM):
  x   [128 p, 16 s, 2 t, 772] bf16 -- position-major, position = t*128+p,
      cols 768:772 hold literal 1.0 (fused softmax denominator)
  xt  [128 dp, 16 s, 6 dc, 2 t, 128 pc] fp8e4m3 -- feature-major transposed
      copy (feature = dc*128+dp), Q/K side only; fp8 only perturbs logits
Output y [128 p, 16 s, 2 t, 772] bf16: cols 0:768 = un-normalized E @ [X|1]
numerator, col 768 = softmax denominator; host divides.

Per segment (L=256, D=768):
  1. batch input DMAs (x on sync HWDGE ring, xt on scalar HWDGE ring)
  2. S^T tiles in PSUM f32 from fp8 matmuls: kt0 full [128,256], kt1 only
     q 128:256 -- S is symmetric, the missing corner E_B[:, 0:128] is the
     PE-transpose of E_A[:, 128:256] after exp
  3. one exp per segment on ScalarE (scale 1/sqrt(768)) -> E bf16 [128,384]
  4. out tiles [128, 384|388] f32 = E[kt][:, qblk].T @ [X[kt] | ones] bf16
  5. plain PSUM->SBUF bf16 evicts (split ScalarE/VectorE), no normalize
  6. per-segment output DMA (gpsimd SWDGE) -- keeps the pipeline tail short
"""

import numpy as np
import ml_dtypes

import concourse.bass as bass
import concourse.mybir as mybir
import concourse.tile as tile
from concourse.bass_utils import run_bass_kernel_spmd
from concourse.masks import make_identity

F32 = mybir.dt.float32
BF16 = mybir.dt.bfloat16
FP8 = mybir.dt.float8e4

B, S_FULL, D = 4, 16384, 768
SEG, DIL = 512, 2
L = SEG // DIL                      # 256 positions per dilated segment
NSEG = B * (S_FULL // SEG)          # 128 segments total
NCORE = 8
SEG_PER_CORE = NSEG // NCORE        # 16
KT = L // 128                       # 2 position tiles per segment
DT = D // 128                       # 6 feature tiles
DW = D + 4                          # free pitch (cols 768:772 = 1.0)
SCALE = 1.0 / float(np.sqrt(D))
MAXB = 2                            # segments per input-DMA batch
TT = MAXB * KT
OW = D + 1                          # output pitch: 768 numerator + denominator


def build_nc():
    nc = bass.Bass()
    x = nc.dram_tensor("x", [128, SEG_PER_CORE, KT, DW], BF16, kind="ExternalInput")
    xt = nc.dram_tensor(
        "xt", [128, SEG_PER_CORE, DT, KT, 128], FP8, kind="ExternalInput"
    )
    y = nc.dram_tensor("y", [128, SEG_PER_CORE, KT, OW], BF16, kind="ExternalOutput")
    Exp = mybir.ActivationFunctionType.Exp

    with tile.TileContext(nc) as tc:
        with (
            tc.tile_pool(name="const", bufs=1) as const_pool,
            tc.tile_pool(name="xn", bufs=3) as xn_pool,
            tc.tile_pool(name="xf", bufs=3) as xf_pool,
            tc.tile_pool(name="e", bufs=8) as e_pool,
            tc.tile_pool(name="osb", bufs=3) as osb_pool,
            tc.tile_pool(name="ps", bufs=2, space="PSUM") as ps_pool,
        ):
            identity_f = const_pool.tile([128, 128], F32)
            make_identity(nc, identity_f[:])
            identity = const_pool.tile([128, 128], BF16)
            nc.vector.tensor_copy(identity[:], identity_f[:])

            batches = [(0, 1), (1, 1)] + [(s, 2) for s in range(2, 16, 2)]
            for s0, bn in batches:
                TB = bn * KT
                xn = xn_pool.tile([128, TT, DW], BF16, tag="xn")
                xf = xf_pool.tile([128, MAXB, DT, KT, 128], FP8, tag="xf")
                nc.sync.dma_start(
                    out=xf[:, 0:bn],
                    in_=xt[:, s0 : s0 + bn],
                )
                nc.sync.dma_start(
                    out=xn[:, 0:TB, :],
                    in_=x[:, s0 : s0 + bn].rearrange("p s t d -> p (s t) d"),
                )

                # ---- Q/K phase for the whole batch
                es_all = []
                for sl in range(bn):
                    sp = ps_pool.tile([128, 384], F32, tag="sp")
                    for d in range(DT):
                        nc.tensor.matmul(
                            sp[:, 0:256],
                            xf[:, sl, d, 0, :],
                            xf[:, sl, d],
                            start=(d == 0),
                            stop=(d == DT - 1),
                        )
                    for d in range(DT):
                        nc.tensor.matmul(
                            sp[:, 256:384],
                            xf[:, sl, d, 1, :],
                            xf[:, sl, d, 1, :],
                            start=(d == 0),
                            stop=(d == DT - 1),
                            skip_group_check=True,
                        )
                    # e[:, 0:256] = E_A; e[:, 256:384] = E_B[:, 128:256],
                    # e[:, 384:512] = E_B[:, 0:128] (transposed corner)
                    e = e_pool.tile([128, 512], BF16)
                    nc.scalar.activation(e[:, 0:384], sp[:], Exp, scale=SCALE)
                    es_all.append(e)

                # ---- V phase + store, output DMA per batch
                osb = osb_pool.tile([128, TT, OW], BF16, tag="osb")
                for sl in range(bn):
                    e = es_all[sl]
                    # S symmetry: E_B[:, 0:128] = (E_A[:, 128:256]).T
                    tpe = ps_pool.tile([128, 128], BF16, tag="tp")
                    nc.tensor.transpose(tpe[:], e[:, 128:256], identity[:])
                    nc.vector.tensor_copy(e[:, 384:512], tpe[:])

                    for qt in (1, 0):  # qt=0 last: it needs the corner evict
                        op0 = ps_pool.tile([128, 388], F32, tag="op0")
                        op1 = ps_pool.tile([128, 388], F32, tag="op1")
                        for kt in range(KT):
                            if kt == 0:
                                lhsT = e[:, qt * 128 : qt * 128 + 128]
                            elif qt == 1:
                                lhsT = e[:, 256:384]
                            else:
                                lhsT = e[:, 384:512]
                            nc.tensor.matmul(
                                op0[:, 0:384],
                                lhsT,
                                xn[:, sl * KT + kt, 0:384],
                                start=(kt == 0),
                                stop=(kt == KT - 1),
                            )
                            nc.tensor.matmul(
                                op1[:, 0:388],
                                lhsT,
                                xn[:, sl * KT + kt, 384:772],
                                start=(kt == 0),
                                stop=(kt == KT - 1),
                            )
                        dst = osb[:, sl * KT + qt]
                        if qt:
                            nc.scalar.copy(dst[:, 0:384], op0[:, 0:384])
                            nc.vector.tensor_copy(dst[:, 384:769], op1[:, 0:385])
                        else:
                            nc.vector.tensor_copy(dst[:, 0:384], op0[:, 0:384])
                            nc.scalar.copy(dst[:, 384:769], op1[:, 0:385])

                nc.gpsimd.dma_start(
                    out=y[:, s0 : s0 + bn].rearrange("p s t d -> p (s t) d"),
                    in_=osb[:, 0:TB],
                )
    return nc


def split_excess_waits(nc, max_waits=1):
    """This walrus build only encodes one sync wait per instruction; move
    excess waits onto preceding same-engine NOPs."""
    n_split = 0
    for fn in nc.m.functions:
        for blk in fn.blocks:
            insts = blk.instructions
            i = 0
            while i < len(insts):
                inst = insts[i]
                si = getattr(inst, "sync_info", None)
                waits = list(si.on_wait) if si and si.on_wait else []
                if len(waits) > max_waits:
                    nop = mybir.InstNoOp(name=f"I-waitsplit-{n_split}", ins=[], outs=[])
                    nop.engine = inst.engine
                    nop.sync_info = mybir.SyncInfo(
                        on_wait=waits[:max_waits], on_update=[]
                    )
                    inst.sync_info = mybir.SyncInfo(
                        on_wait=waits[max_waits:], on_update=list(si.on_update)
                    )
                    insts.insert(i, nop)
                    n_split += 1
                else:
                    i += 1
    return n_split


_NC = None


def _get_nc():
    global _NC
    if _NC is None:
        _NC = build_nc()
        split_excess_waits(_NC)
    return _NC


def shard_inputs(x):
    """Full x [4, 16384, 768] f32 -> 8 per-core dicts:
    x  [128, 16, 2, 772] bf16 (position-major + ones cols)
    xt [128, 16, 6, 2, 128] fp8e4m3 (feature-major)
    """
    xd = np.asarray(x).reshape(B, S_FULL // SEG, SEG, D)[:, :, ::DIL, :]
    xd = xd.reshape(NSEG, KT, 128, D)                 # [seg, t, p, d]
    xp = xd.transpose(2, 0, 1, 3)                     # [p, seg, t, d]
    xb = np.empty((128, NSEG, KT, DW), dtype=ml_dtypes.bfloat16)
    xb[..., 0:D] = xp.astype(ml_dtypes.bfloat16)
    xb[..., D:DW] = np.asarray(1.0, dtype=ml_dtypes.bfloat16)
    xt = (
        xb[..., 0:D]
        .reshape(128, NSEG, KT, DT, 128)              # [p, seg, t, dc, dp]
        .transpose(4, 1, 3, 2, 0)                     # [dp, seg, dc, t, p]
        .astype(ml_dtypes.float8_e4m3)
    )
    out = []
    for c in range(NCORE):
        sl = slice(SEG_PER_CORE * c, SEG_PER_CORE * (c + 1))
        out.append(
            {
                "x": np.ascontiguousarray(xb[:, sl]),
                "xt": np.ascontiguousarray(xt[:, sl]),
            }
        )
    return out


def assemble_output(results):
    ys = np.concatenate([results[c]["y"] for c in range(NCORE)], axis=1)
    ys = ys.astype(np.float32)                        # [p, seg, t, 769]
    num = ys[..., 0:D].transpose(1, 2, 0, 3)          # [seg, t, p, d]
    den = ys[..., D].transpose(1, 2, 0)[..., None]    # [seg, t, p, 1]
    out = num / den
    return np.ascontiguousarray(out.reshape(B, (S_FULL // SEG) * L, D))


def kernel(x):
    nc = _get_nc()
    in_maps = shard_inputs(x)
    core_ids = list(range(NCORE))
    # run twice: the first execution after a fresh NEFF load has been seen
    # returning unwritten output buffers; the repeat is cheap and reliable.
    run_bass_kernel_spmd(nc, in_maps, core_ids)
    res = run_bass_kernel_spmd(nc, in_maps, core_ids)
    return assemble_output(res.results)


# revision 17
# speedup vs baseline: 1.9440x; 1.0416x over previous
"""Dilated-attention Trainium2 kernel (8 NeuronCores, SPMD), bf16/fp8 edition.

Problem: x [4, 16384, 768] f32. Per 512-token segment, take every 2nd
position (dilation 2) -> 128 independent segments of [256, 768]; per-segment
self-attention out = softmax(xs @ xs.T / sqrt(768)) @ xs; output [4, 8192, 768].

Sharding: 128 (batch x segment) attention problems are fully independent ->
16 segments per core, no cross-core communication. The dilation gather, the
position-major -> partition-major permutation, the bf16/fp8 casts and the
final numerator/denominator divide are host-side (pure data movement /
elementwise; overall relative error ~2.3e-3, well under the 2e-2 gate).

Device inputs per core (all per-partition contiguous in DRAM):
  x   [128 p, 16 s, 2 t, 772] bf16 -- position-major, position = t*128+p,
      cols 768:772 hold literal 1.0 (fused softmax denominator)
  xt  [128 dp, 16 s, 6 dc, 2 t, 128 pc] fp8e4m3 -- feature-major transposed
      copy (feature = dc*128+dp), Q/K side only; fp8 only perturbs logits
Output y [128 p, 16 s, 2 t, 772] bf16: cols 0:768 = un-normalized E @ [X|1]
numerator, col 768 = softmax denominator; host divides.

Per segment (L=256, D=768):
  1. batch input DMAs (x on sync HWDGE ring, xt on scalar HWDGE ring)
  2. S^T tiles in PSUM f32 from fp8 matmuls: kt0 full [128,256], kt1 only
     q 128:256 -- S is symmetric, the missing corner E_B[:, 0:128] is the
     PE-transpose of E_A[:, 128:256] after exp
  3. one exp per segment on ScalarE (scale 1/sqrt(768)) -> E bf16 [128,384]
  4. out tiles [128, 384|388] f32 = E[kt][:, qblk].T @ [X[kt] | ones] bf16
  5. plain PSUM->SBUF bf16 evicts (split ScalarE/VectorE), no normalize
  6. per-segment output DMA (gpsimd SWDGE) -- keeps the pipeline tail short
"""

import numpy as np
import ml_dtypes

import concourse.bass as bass
import concourse.mybir as mybir
import concourse.tile as tile
from concourse.bass_utils import run_bass_kernel_spmd
from concourse.masks import make_identity

F32 = mybir.dt.float32
BF16 = mybir.dt.bfloat16
FP8 = mybir.dt.float8e4

B, S_FULL, D = 4, 16384, 768
SEG, DIL = 512, 2
L = SEG // DIL                      # 256 positions per dilated segment
NSEG = B * (S_FULL // SEG)          # 128 segments total
NCORE = 8
SEG_PER_CORE = NSEG // NCORE        # 16
KT = L // 128                       # 2 position tiles per segment
DT = D // 128                       # 6 feature tiles
DW = D + 4                          # free pitch (cols 768:772 = 1.0)
SCALE = 1.0 / float(np.sqrt(D))
MAXB = 2                            # segments per input-DMA batch
TT = MAXB * KT
OW = D + 1                          # output pitch: 768 numerator + denominator


def build_nc():
    nc = bass.Bass()
    x = nc.dram_tensor("x", [128, SEG_PER_CORE, KT, DW], BF16, kind="ExternalInput")
    xt = nc.dram_tensor(
        "xt", [128, SEG_PER_CORE, DT, KT, 128], FP8, kind="ExternalInput"
    )
    y = nc.dram_tensor("y", [128, SEG_PER_CORE, KT, OW], BF16, kind="ExternalOutput")
    Exp = mybir.ActivationFunctionType.Exp

    with tile.TileContext(nc) as tc:
        with (
            tc.tile_pool(name="const", bufs=1) as const_pool,
            tc.tile_pool(name="xn", bufs=4) as xn_pool,
            tc.tile_pool(name="xf", bufs=4) as xf_pool,
            tc.tile_pool(name="e", bufs=8) as e_pool,
            tc.tile_pool(name="osb", bufs=3) as osb_pool,
            tc.tile_pool(name="ps", bufs=2, space="PSUM") as ps_pool,
        ):
            identity_f = const_pool.tile([128, 128], F32)
            make_identity(nc, identity_f[:])
            identity = const_pool.tile([128, 128], BF16)
            nc.vector.tensor_copy(identity[:], identity_f[:])

            batches = [(0, 1), (1, 1)] + [(s, 2) for s in range(2, 16, 2)]
            LOOKAHEAD = 2

            def emit_dma(bi):
                s0, bn = batches[bi]
                TB = bn * KT
                xn = xn_pool.tile([128, TT, DW], BF16, tag="xn")
                xf = xf_pool.tile([128, MAXB, DT, KT, 128], FP8, tag="xf")
                nc.sync.dma_start(out=xf[:, 0:bn], in_=xt[:, s0 : s0 + bn])
                nc.scalar.dma_start(
                    out=xn[:, 0:TB, :],
                    in_=x[:, s0 : s0 + bn].rearrange("p s t d -> p (s t) d"),
                )
                return xn, xf

            dmas = [emit_dma(i) for i in range(1 + LOOKAHEAD)]
            for bi, (s0, bn) in enumerate(batches):
                TB = bn * KT
                xn, xf = dmas[bi]

                # ---- Q/K phase for the whole batch
                es_all = []
                for sl in range(bn):
                    sp = ps_pool.tile([128, 384], F32, tag="sp")
                    for d in range(DT):
                        nc.tensor.matmul(
                            sp[:, 0:256],
                            xf[:, sl, d, 0, :],
                            xf[:, sl, d],
                            start=(d == 0),
                            stop=(d == DT - 1),
                        )
                    for d in range(DT):
                        nc.tensor.matmul(
                            sp[:, 256:384],
                            xf[:, sl, d, 1, :],
                            xf[:, sl, d, 1, :],
                            start=(d == 0),
                            stop=(d == DT - 1),
                            skip_group_check=True,
                        )
                    # e[:, 0:256] = E_A; e[:, 256:384] = E_B[:, 128:256],
                    # e[:, 384:512] = E_B[:, 0:128] (transposed corner)
                    e = e_pool.tile([128, 512], BF16)
                    nc.scalar.activation(e[:, 0:384], sp[:], Exp, scale=SCALE)
                    es_all.append(e)

                # ---- V phase + store, output DMA per batch
                osb = osb_pool.tile([128, TT, OW], BF16, tag="osb")
                for sl in range(bn):
                    e = es_all[sl]
                    # S symmetry: E_B[:, 0:128] = (E_A[:, 128:256]).T
                    tpe = ps_pool.tile([128, 128], BF16, tag="tp")
                    nc.tensor.transpose(tpe[:], e[:, 128:256], identity[:])
                    nc.vector.tensor_copy(e[:, 384:512], tpe[:])

                    for qt in (1, 0):  # qt=0 last: it needs the corner evict
                        op0 = ps_pool.tile([128, 388], F32, tag="op0")
                        op1 = ps_pool.tile([128, 388], F32, tag="op1")
                        for kt in range(KT):
                            if kt == 0:
                                lhsT = e[:, qt * 128 : qt * 128 + 128]
                            elif qt == 1:
                                lhsT = e[:, 256:384]
                            else:
                                lhsT = e[:, 384:512]
                            nc.tensor.matmul(
                                op0[:, 0:384],
                                lhsT,
                                xn[:, sl * KT + kt, 0:384],
                                start=(kt == 0),
                                stop=(kt == KT - 1),
                            )
                            nc.tensor.matmul(
                                op1[:, 0:388],
                                lhsT,
                                xn[:, sl * KT + kt, 384:772],
                                start=(kt == 0),
                                stop=(kt == KT - 1),
                            )
                        dst = osb[:, sl * KT + qt]
                        if qt:
                            nc.scalar.copy(dst[:, 0:384], op0[:, 0:384])
                            nc.vector.tensor_copy(dst[:, 384:769], op1[:, 0:385])
                        else:
                            nc.vector.tensor_copy(dst[:, 0:384], op0[:, 0:384])
                            nc.scalar.copy(dst[:, 384:769], op1[:, 0:385])

                if bi + 1 + LOOKAHEAD < len(batches):
                    dmas.append(emit_dma(bi + 1 + LOOKAHEAD))

                if bi == len(batches) - 1 and bn == 2:
                    # split the final store across two queues: short tail
                    nc.gpsimd.dma_start(
                        out=y[:, s0].rearrange("p t d -> p (t d)"),
                        in_=osb[:, 0:KT].rearrange("p t d -> p (t d)"),
                    )
                    nc.scalar.dma_start(
                        out=y[:, s0 + 1].rearrange("p t d -> p (t d)"),
                        in_=osb[:, KT : 2 * KT].rearrange("p t d -> p (t d)"),
                    )
                else:
                    nc.gpsimd.dma_start(
                        out=y[:, s0 : s0 + bn].rearrange("p s t d -> p (s t) d"),
                        in_=osb[:, 0:TB],
                    )
    return nc


def split_excess_waits(nc, max_waits=1):
    """This walrus build only encodes one sync wait per instruction; move
    excess waits onto preceding same-engine NOPs."""
    n_split = 0
    for fn in nc.m.functions:
        for blk in fn.blocks:
            insts = blk.instructions
            i = 0
            while i < len(insts):
                inst = insts[i]
                si = getattr(inst, "sync_info", None)
                waits = list(si.on_wait) if si and si.on_wait else []
                if len(waits) > max_waits:
                    nop = mybir.InstNoOp(name=f"I-waitsplit-{n_split}", ins=[], outs=[])
                    nop.engine = inst.engine
                    nop.sync_info = mybir.SyncInfo(
                        on_wait=waits[:max_waits], on_update=[]
                    )
                    inst.sync_info = mybir.SyncInfo(
                        on_wait=waits[max_waits:], on_update=list(si.on_update)
                    )
                    insts.insert(i, nop)
                    n_split += 1
                else:
                    i += 1
    return n_split


_NC = None


def _get_nc():
    global _NC
    if _NC is None:
        _NC = build_nc()
        split_excess_waits(_NC)
    return _NC


def shard_inputs(x):
    """Full x [4, 16384, 768] f32 -> 8 per-core dicts:
    x  [128, 16, 2, 772] bf16 (position-major + ones cols)
    xt [128, 16, 6, 2, 128] fp8e4m3 (feature-major)
    """
    xd = np.asarray(x).reshape(B, S_FULL // SEG, SEG, D)[:, :, ::DIL, :]
    xd = xd.reshape(NSEG, KT, 128, D)                 # [seg, t, p, d]
    xp = xd.transpose(2, 0, 1, 3)                     # [p, seg, t, d]
    xb = np.empty((128, NSEG, KT, DW), dtype=ml_dtypes.bfloat16)
    xb[..., 0:D] = xp.astype(ml_dtypes.bfloat16)
    xb[..., D:DW] = np.asarray(1.0, dtype=ml_dtypes.bfloat16)
    xt = (
        xb[..., 0:D]
        .reshape(128, NSEG, KT, DT, 128)              # [p, seg, t, dc, dp]
        .transpose(4, 1, 3, 2, 0)                     # [dp, seg, dc, t, p]
        .astype(ml_dtypes.float8_e4m3)
    )
    out = []
    for c in range(NCORE):
        sl = slice(SEG_PER_CORE * c, SEG_PER_CORE * (c + 1))
        out.append(
            {
                "x": np.ascontiguousarray(xb[:, sl]),
                "xt": np.ascontiguousarray(xt[:, sl]),
            }
        )
    return out


def assemble_output(results):
    ys = np.concatenate([results[c]["y"] for c in range(NCORE)], axis=1)
    ys = ys.astype(np.float32)                        # [p, seg, t, 769]
    num = ys[..., 0:D].transpose(1, 2, 0, 3)          # [seg, t, p, d]
    den = ys[..., D].transpose(1, 2, 0)[..., None]    # [seg, t, p, 1]
    out = num / den
    return np.ascontiguousarray(out.reshape(B, (S_FULL // SEG) * L, D))


def kernel(x):
    nc = _get_nc()
    in_maps = shard_inputs(x)
    core_ids = list(range(NCORE))
    # run twice: the first execution after a fresh NEFF load has been seen
    # returning unwritten output buffers; the repeat is cheap and reliable.
    run_bass_kernel_spmd(nc, in_maps, core_ids)
    res = run_bass_kernel_spmd(nc, in_maps, core_ids)
    return assemble_output(res.results)


# revision 22
# speedup vs baseline: 1.9873x; 1.0223x over previous
"""Dilated-attention Trainium2 kernel (8 NeuronCores, SPMD), bf16/fp8 edition.

Problem: x [4, 16384, 768] f32. Per 512-token segment, take every 2nd
position (dilation 2) -> 128 independent segments of [256, 768]; per-segment
self-attention out = softmax(xs @ xs.T / sqrt(768)) @ xs; output [4, 8192, 768].

Sharding: 128 (batch x segment) attention problems are fully independent ->
16 segments per core, no cross-core communication. The dilation gather, the
position-major -> partition-major permutation, the bf16/fp8 casts and the
final numerator/denominator divide are host-side (pure data movement /
elementwise; overall relative error ~2.3e-3, well under the 2e-2 gate).

Device inputs per core (all per-partition contiguous in DRAM):
  x   [128 p, 16 s, 2 t, 772] bf16 -- position-major, position = t*128+p,
      cols 768:772 hold literal 1.0 (fused softmax denominator)
  xt  [128 dp, 16 s, 6 dc, 2 t, 128 pc] fp8e4m3 -- feature-major transposed
      copy (feature = dc*128+dp), Q/K side only; fp8 only perturbs logits
Output y [128 p, 16 s, 2 t, 772] bf16: cols 0:768 = un-normalized E @ [X|1]
numerator, col 768 = softmax denominator; host divides.

Per segment (L=256, D=768):
  1. batch input DMAs (x on sync HWDGE ring, xt on scalar HWDGE ring)
  2. S^T tiles in PSUM f32 from fp8 matmuls: kt0 full [128,256], kt1 only
     q 128:256 -- S is symmetric, the missing corner E_B[:, 0:128] is the
     PE-transpose of E_A[:, 128:256] after exp
  3. one exp per segment on ScalarE (scale 1/sqrt(768)) -> E bf16 [128,384]
  4. out tiles [128, 384|388] f32 = E[kt][:, qblk].T @ [X[kt] | ones] bf16
  5. plain PSUM->SBUF bf16 evicts (split ScalarE/VectorE), no normalize
  6. per-segment output DMA (gpsimd SWDGE) -- keeps the pipeline tail short
"""

import numpy as np
import ml_dtypes

import concourse.bass as bass
import concourse.mybir as mybir
import concourse.tile as tile
from concourse.bass_utils import run_bass_kernel_spmd
from concourse.masks import make_identity

F32 = mybir.dt.float32
BF16 = mybir.dt.bfloat16
FP8 = mybir.dt.float8e4

B, S_FULL, D = 4, 16384, 768
SEG, DIL = 512, 2
L = SEG // DIL                      # 256 positions per dilated segment
NSEG = B * (S_FULL // SEG)          # 128 segments total
NCORE = 8
SEG_PER_CORE = NSEG // NCORE        # 16
KT = L // 128                       # 2 position tiles per segment
DT = D // 128                       # 6 feature tiles
DW = D + 4                          # free pitch (cols 768:772 = 1.0)
SCALE = 1.0 / float(np.sqrt(D))
MAXB = 2                            # segments per input-DMA batch
TT = MAXB * KT
OW = D + 1                          # output pitch: 768 numerator + denominator


def build_nc():
    nc = bass.Bass()
    x = nc.dram_tensor("x", [128, SEG_PER_CORE, KT, DW], BF16, kind="ExternalInput")
    # DoubleRow-interleaved feature-major copy: [dp, s, j, c, pos],
    # feature = j*256 + c*128 + dp (virtual 256-deep contraction per matmul)
    xt = nc.dram_tensor(
        "xt", [128, SEG_PER_CORE, DT // 2, 2, L], FP8, kind="ExternalInput"
    )
    y = nc.dram_tensor("y", [128, SEG_PER_CORE, KT, OW], BF16, kind="ExternalOutput")
    Exp = mybir.ActivationFunctionType.Exp

    with tile.TileContext(nc) as tc:
        with (
            tc.tile_pool(name="const", bufs=1) as const_pool,
            tc.tile_pool(name="xn", bufs=4) as xn_pool,
            tc.tile_pool(name="xf", bufs=4) as xf_pool,
            tc.tile_pool(name="e", bufs=8) as e_pool,
            tc.tile_pool(name="osb", bufs=3) as osb_pool,
            tc.tile_pool(name="ps", bufs=2, space="PSUM") as ps_pool,
        ):
            identity_f = const_pool.tile([128, 128], F32)
            make_identity(nc, identity_f[:])
            identity = const_pool.tile([128, 128], BF16)
            nc.vector.tensor_copy(identity[:], identity_f[:])

            batches = [(0, 1), (1, 1)] + [(s, 2) for s in range(2, 16, 2)]
            LOOKAHEAD = 2

            def emit_dma(bi):
                s0, bn = batches[bi]
                TB = bn * KT
                xn = xn_pool.tile([128, TT, DW], BF16, tag="xn")
                xf = xf_pool.tile([128, MAXB, DT // 2, 2, L], FP8, tag="xf")
                nc.sync.dma_start(out=xf[:, 0:bn], in_=xt[:, s0 : s0 + bn])
                nc.scalar.dma_start(
                    out=xn[:, 0:TB, :],
                    in_=x[:, s0 : s0 + bn].rearrange("p s t d -> p (s t) d"),
                )
                return xn, xf

            dmas = [emit_dma(i) for i in range(1 + LOOKAHEAD)]
            for bi, (s0, bn) in enumerate(batches):
                TB = bn * KT
                xn, xf = dmas[bi]

                # ---- Q/K phase for the whole batch
                es_all = []
                for sl in range(bn):
                    sp = ps_pool.tile([128, 384], F32, tag="sp")
                    DR = mybir.MatmulPerfMode.DoubleRow
                    for j in range(DT // 2):
                        nc.tensor.matmul(
                            sp[:, 0:256],
                            xf[:, sl, j, :, 0:128],
                            xf[:, sl, j],
                            start=(j == 0),
                            stop=(j == DT // 2 - 1),
                            perf_mode=DR,
                        )
                    for j in range(DT // 2):
                        nc.tensor.matmul(
                            sp[:, 256:384],
                            xf[:, sl, j, :, 128:256],
                            xf[:, sl, j, :, 128:256],
                            start=(j == 0),
                            stop=(j == DT // 2 - 1),
                            perf_mode=DR,
                            skip_group_check=True,
                        )
                    # e[:, 0:256] = E_A; e[:, 256:384] = E_B[:, 128:256],
                    # e[:, 384:512] = E_B[:, 0:128] (transposed corner)
                    e = e_pool.tile([128, 512], BF16)
                    nc.scalar.activation(e[:, 0:384], sp[:], Exp, scale=SCALE)
                    es_all.append(e)

                # ---- V phase + store, output DMA per batch
                osb = osb_pool.tile([128, TT, OW], BF16, tag="osb")
                for sl in range(bn):
                    e = es_all[sl]
                    # S symmetry: E_B[:, 0:128] = (E_A[:, 128:256]).T
                    tpe = ps_pool.tile([128, 128], BF16, tag="tp")
                    nc.tensor.transpose(tpe[:], e[:, 128:256], identity[:])
                    nc.vector.tensor_copy(e[:, 384:512], tpe[:])

                    for qt in (1, 0):  # qt=0 last: it needs the corner evict
                        op0 = ps_pool.tile([128, 388], F32, tag="op0")
                        op1 = ps_pool.tile([128, 388], F32, tag="op1")
                        for kt in range(KT):
                            if kt == 0:
                                lhsT = e[:, qt * 128 : qt * 128 + 128]
                            elif qt == 1:
                                lhsT = e[:, 256:384]
                            else:
                                lhsT = e[:, 384:512]
                            nc.tensor.matmul(
                                op0[:, 0:384],
                                lhsT,
                                xn[:, sl * KT + kt, 0:384],
                                start=(kt == 0),
                                stop=(kt == KT - 1),
                            )
                            nc.tensor.matmul(
                                op1[:, 0:388],
                                lhsT,
                                xn[:, sl * KT + kt, 384:772],
                                start=(kt == 0),
                                stop=(kt == KT - 1),
                            )
                        dst = osb[:, sl * KT + qt]
                        if qt:
                            nc.scalar.copy(dst[:, 0:384], op0[:, 0:384])
                            nc.vector.tensor_copy(dst[:, 384:769], op1[:, 0:385])
                        else:
                            nc.vector.tensor_copy(dst[:, 0:384], op0[:, 0:384])
                            nc.scalar.copy(dst[:, 384:769], op1[:, 0:385])

                if bi + 1 + LOOKAHEAD < len(batches):
                    dmas.append(emit_dma(bi + 1 + LOOKAHEAD))

                if bi == len(batches) - 1 and bn == 2:
                    # split the final store across two queues: short tail
                    nc.gpsimd.dma_start(
                        out=y[:, s0].rearrange("p t d -> p (t d)"),
                        in_=osb[:, 0:KT].rearrange("p t d -> p (t d)"),
                    )
                    nc.scalar.dma_start(
                        out=y[:, s0 + 1].rearrange("p t d -> p (t d)"),
                        in_=osb[:, KT : 2 * KT].rearrange("p t d -> p (t d)"),
                    )
                else:
                    eng = nc.scalar if bi % 2 else nc.sync
                    eng.dma_start(
                        out=y[:, s0 : s0 + bn].rearrange("p s t d -> p (s t) d"),
                        in_=osb[:, 0:TB],
                    )
    return nc


def split_excess_waits(nc, max_waits=1):
    """This walrus build only encodes one sync wait per instruction; move
    excess waits onto preceding same-engine NOPs."""
    n_split = 0
    for fn in nc.m.functions:
        for blk in fn.blocks:
            insts = blk.instructions
            i = 0
            while i < len(insts):
                inst = insts[i]
                si = getattr(inst, "sync_info", None)
                waits = list(si.on_wait) if si and si.on_wait else []
                if len(waits) > max_waits:
                    nop = mybir.InstNoOp(name=f"I-waitsplit-{n_split}", ins=[], outs=[])
                    nop.engine = inst.engine
                    nop.sync_info = mybir.SyncInfo(
                        on_wait=waits[:max_waits], on_update=[]
                    )
                    inst.sync_info = mybir.SyncInfo(
                        on_wait=waits[max_waits:], on_update=list(si.on_update)
                    )
                    insts.insert(i, nop)
                    n_split += 1
                else:
                    i += 1
    return n_split


_NC = None


def _get_nc():
    global _NC
    if _NC is None:
        _NC = build_nc()
        split_excess_waits(_NC)
    return _NC


def shard_inputs(x):
    """Full x [4, 16384, 768] f32 -> 8 per-core dicts:
    x  [128, 16, 2, 772] bf16 (position-major + ones cols)
    xt [128, 16, 6, 2, 128] fp8e4m3 (feature-major)
    """
    xd = np.asarray(x).reshape(B, S_FULL // SEG, SEG, D)[:, :, ::DIL, :]
    xd = xd.reshape(NSEG, KT, 128, D)                 # [seg, t, p, d]
    xp = xd.transpose(2, 0, 1, 3)                     # [p, seg, t, d]
    xb = np.empty((128, NSEG, KT, DW), dtype=ml_dtypes.bfloat16)
    xb[..., 0:D] = xp.astype(ml_dtypes.bfloat16)
    xb[..., D:DW] = np.asarray(1.0, dtype=ml_dtypes.bfloat16)
    xt = (
        xb[..., 0:D]
        .reshape(128, NSEG, KT, DT // 2, 2, 128)      # [p, seg, t, j, c, dp]
        .transpose(5, 1, 3, 4, 2, 0)                  # [dp, seg, j, c, t, p]
        .reshape(128, NSEG, DT // 2, 2, L)            # [dp, seg, j, c, pos]
        .astype(ml_dtypes.float8_e4m3)
    )
    out = []
    for c in range(NCORE):
        sl = slice(SEG_PER_CORE * c, SEG_PER_CORE * (c + 1))
        out.append(
            {
                "x": np.ascontiguousarray(xb[:, sl]),
                "xt": np.ascontiguousarray(xt[:, sl]),
            }
        )
    return out


def assemble_output(results):
    ys = np.concatenate([results[c]["y"] for c in range(NCORE)], axis=1)
    ys = ys.astype(np.float32)                        # [p, seg, t, 769]
    num = ys[..., 0:D].transpose(1, 2, 0, 3)          # [seg, t, p, d]
    den = ys[..., D].transpose(1, 2, 0)[..., None]    # [seg, t, p, 1]
    out = num / den
    return np.ascontiguousarray(out.reshape(B, (S_FULL // SEG) * L, D))


def kernel(x):
    nc = _get_nc()
    in_maps = shard_inputs(x)
    core_ids = list(range(NCORE))
    # run twice: the first execution after a fresh NEFF load has been seen
    # returning unwritten output buffers; the repeat is cheap and reliable.
    run_bass_kernel_spmd(nc, in_maps, core_ids)
    res = run_bass_kernel_spmd(nc, in_maps, core_ids)
    return assemble_output(res.results)
